# revision 26
# baseline (speedup 1.0000x reference)
"""Code2Vec forward kernel for Trainium2 (Bass/Tile), data-parallel over batch.

Model (per batch row b):
  es = node_emb[starts[b]]; ep = path_emb[paths[b]]; ee = node_emb[ends[b]]
  x  = tanh([es|ep|ee] @ W.T)            # [T, E]
  z  = softmax(x @ a)                    # [T], over full T
  v  = sum_t x[t] * (z*mask)[t]          # [E]
  out = v @ out_W.T + out_b              # [OUT]

Sharding: 8 NeuronCores, 8 batch rows each; embedding tables replicated.

Perf notes vs the first working version (184.5us):
- fp16 tables/weights/matmuls (rel err ~3e-3, gate is 2e-2): PE matmuls and
  transposes run 4x faster than fp32.
- Gather dest rows are PADDED to 512B: 256B gather descriptors that share a
  512B SBUF line go through the sub-512B read-modify-write path and corrupt
  nondeterministically when executed by different SDMA engines.
- Multi-offset (batched) indirect DMAs and tensor_tensor_reduce are broken
  on HW (wrong data / device wedge) -- do not use. dma_gather works but
  costs the same ~8.4ns/row of Q7 time and caps at 1024 indices/call; with
  the extra reorder pass it does not beat per-j indirect gathers.
"""

import os
import sys

import numpy as np

sys.path.insert(0, "/opt/trn_rl_repo")

B, T, E = 64, 512, 128
NODES, PATHS, OUT = 100000, 200000, 1000
PAD = 1
NCORES = 8
BC = B // NCORES          # batch rows per core
CHUNKS = T // 128         # 128-token chunks per batch row
J = BC * CHUNKS           # token tiles per core (32)
NG = int(os.environ.get("KNG", "32"))  # gather splits per table (32 = per-j)
JG = J // NG

_BUILT = None
LAST_RESULTS = None
TRACE = False


def _build():
    """Build the (SPMD, identical across cores) Bass kernel once."""
    from contextlib import ExitStack

    import concourse.bacc as bacc
    import concourse.bass as bass
    import concourse.tile as tile
    from concourse import mybir

    f32 = mybir.dt.float32
    f16 = mybir.dt.float16
    i32 = mybir.dt.int32

    nc = bacc.Bacc("TRN2", target_bir_lowering=False, debug=False, num_devices=NCORES)

    d_sidx = nc.dram_tensor("s_idx", [128, J], i32, kind="ExternalInput")
    d_pidx = nc.dram_tensor("p_idx", [128, J], i32, kind="ExternalInput")
    d_eidx = nc.dram_tensor("e_idx", [128, J], i32, kind="ExternalInput")
    d_node = nc.dram_tensor("node_emb", [NODES + 1, E], f16, kind="ExternalInput")
    d_path = nc.dram_tensor("path_emb", [PATHS + 1, E], f16, kind="ExternalInput")
    d_wt = nc.dram_tensor("wt", [128, 3, E], f16, kind="ExternalInput")
    d_aoh = nc.dram_tensor("a_oh", [E, BC * BC], f16, kind="ExternalInput")
    d_ohr = nc.dram_tensor("oh_rows", [128, BC * 128], f16, kind="ExternalInput")
    d_mask = nc.dram_tensor("mask", [36, T], f32, kind="ExternalInput")
    d_owt = nc.dram_tensor("out_wt", [E, OUT], f16, kind="ExternalInput")
    d_ob = nc.dram_tensor("out_b", [BC, OUT], f32, kind="ExternalInput")
    d_ident = nc.dram_tensor("ident", [128, 128], f16, kind="ExternalInput")
    d_out = nc.dram_tensor("out", [BC, OUT], f32, kind="ExternalOutput")


    with ExitStack() as ctx:
        tc = ctx.enter_context(tile.TileContext(nc))
        const = ctx.enter_context(tc.tile_pool(name="const", bufs=1))
        gath = ctx.enter_context(tc.tile_pool(name="gath", bufs=1))
        ctp = ctx.enter_context(tc.tile_pool(name="ct", bufs=2))
        xtp = ctx.enter_context(tc.tile_pool(name="xt", bufs=BC))
        scrp = ctx.enter_context(tc.tile_pool(name="scr", bufs=2))
        smallp = ctx.enter_context(tc.tile_pool(name="small", bufs=1))
        p_tr = ctx.enter_context(tc.tile_pool(name="ptr", bufs=2, space="PSUM"))
        p_x = ctx.enter_context(tc.tile_pool(name="px", bufs=2, space="PSUM"))
        p_s = ctx.enter_context(tc.tile_pool(name="ps", bufs=1, space="PSUM"))

        # ---- small inputs; idx tiles FIRST so the gather stream (the
        # critical path: ~1.4us of Q7 time per call x 96) starts ~12us
        # earlier instead of queueing behind the const loads ----
        sidx_sb = const.tile([128, J], i32)
        nc.sync.dma_start(out=sidx_sb[:], in_=d_sidx[:])
        pidx_sb = const.tile([128, J], i32)
        nc.sync.dma_start(out=pidx_sb[:], in_=d_pidx[:])
        eidx_sb = const.tile([128, J], i32)
        nc.sync.dma_start(out=eidx_sb[:], in_=d_eidx[:])

        ident = const.tile([128, 128], f16)
        nc.sync.dma_start(out=ident[:], in_=d_ident[:])
        wt_sb = const.tile([128, 3, E], f16)
        nc.sync.dma_start(out=wt_sb[:], in_=d_wt[:])
        aoh_sb = const.tile([E, BC * BC], f16)
        nc.sync.dma_start(out=aoh_sb[:], in_=d_aoh[:])
        ohr_sb = const.tile([128, BC * 128], f16)
        nc.sync.dma_start(out=ohr_sb[:], in_=d_ohr[:])
        mask_sb = const.tile([36, T], f32)
        nc.sync.dma_start(out=mask_sb[:], in_=d_mask[:])
        owt_sb = const.tile([E, OUT], f16)
        nc.sync.dma_start(out=owt_sb[:], in_=d_owt[:])
        ob_sb = const.tile([BC, OUT], f32)
        nc.sync.dma_start(out=ob_sb[:], in_=d_ob[:])

        # ---- gathers: indirect DMAs, one 128-row call per (table, j) ----
        # g_*[p, j, 0:E] = table[idx[p, j], :].  Each descriptor transfers a
        # FULL 512B line (rows idx and idx+1; tables carry one extra row so
        # idx+1 stays in bounds): 256B descriptors corrupt nondeterministically
        # via the sub-512B SBUF read-modify-write path when lines are shared,
        # and even padded they halve SDMA write throughput.
        g_es = gath.tile([128, J, 2 * E], f16)
        g_ep = gath.tile([128, J, 2 * E], f16)
        g_ee = gath.tile([128, J, 2 * E], f16)
        for h in range(NG):
            jlo, jhi = h * JG, (h + 1) * JG
            for g, idx, table in (
                (g_es, sidx_sb, d_node),
                (g_ep, pidx_sb, d_path),
                (g_ee, eidx_sb, d_node),
            ):
                for j in range(jlo, jhi):
                    nc.gpsimd.indirect_dma_start(
                        out=g[:, j, 0:2 * E],
                        out_offset=None,
                        in_=table[:],
                        in_offset=bass.IndirectOffsetOnAxis(
                            ap=idx[:, j:j + 1], axis=0
                        ),
                    )

        # ---- per-batch-row pipeline, two groups of 4 rows ----
        # Group A (rows 0-3) finishes scores/softmax/v mid-gather-stream,
        # overlapping group B's gathers; only group B's chain sits in the
        # tail after the last gather lands.  Group B's scores accumulate at
        # PSUM base partition 32 (bases must be 0/32/64) so its softmax and
        # mask rows stay lane-aligned.
        wfp = smallp.tile([128, T], f16, tag="wfp")
        nc.vector.memset(wfp[:], 0.0)
        vt_sb = smallp.tile([128, BC], f32, tag="vt")
        xt_tiles = []
        for gi in range(2):
            rows = range(gi * 4, gi * 4 + 4)
            for b in rows:
                jbase = CHUNKS * b
                # transpose gathered [t, d] chunks -> cT[d, table, t]
                ct = ctp.tile([128, 3, T], f16, tag="ct")
                for c in range(CHUNKS):
                    tr = p_tr.tile([128, 3, 128], f16, tag="tr")
                    for k, g in enumerate((g_es, g_ep, g_ee)):
                        nc.tensor.transpose(
                            out=tr[:, k, :],
                            in_=g[:, jbase + c, 0:E],
                            identity=ident[:],
                        )
                    nc.vector.tensor_copy(
                        out=ct[:, :, c * 128:(c + 1) * 128], in_=tr[:]
                    )
                # x^T[e, t] = sum_k wt[:,k,:].T @ cT[:,k,:]
                px = p_x.tile([128, T], f32, tag="x")
                for k in range(3):
                    nc.tensor.matmul(
                        out=px[:],
                        lhsT=wt_sb[:, k, :],
                        rhs=ct[:, k, :],
                        start=(k == 0),
                        stop=(k == 2),
                    )
                xt = xtp.tile([128, T], f16, tag="xt")
                nc.scalar.activation(
                    out=xt[:], in_=px[:], func=mybir.ActivationFunctionType.Tanh
                )
                xt_tiles.append(xt)

            # scores for this group: S[m, t] = a . xt_{gi*4+m}[:, t]
            # (back-to-back matmuls, accumulation group never left open
            # across other PE work -- that corrupts nondeterministically)
            if gi == 0:
                S_g = p_s.tile([4, T], f32, tag="s")
                pbase, cofs = 0, 0
            else:
                S_g = p_s.tile([36, T], f32, tag="s2")
                pbase, cofs = 32, 4
            sl = slice(pbase, pbase + 4)
            for b in rows:
                nc.tensor.matmul(
                    out=S_g[sl, :] if gi else S_g[:],
                    lhsT=aoh_sb[:, b * BC + cofs:b * BC + cofs + 4],
                    rhs=xt_tiles[b][:],
                    start=(b == rows[0]),
                    stop=(b == rows[-1]),
                )
            Sv = S_g[sl, :] if gi else S_g[:]

            # masked softmax over t (free dim) for this group's 4 rows
            nm = smallp.tile([36, 1], f32, tag=f"negmax{gi}")
            nc.vector.tensor_reduce(
                out=nm[sl, :], in_=Sv, axis=mybir.AxisListType.X,
                op=mybir.AluOpType.max, negate=True,
            )
            ex = smallp.tile([36, T], f32, tag=f"ex{gi}")
            nc.scalar.activation(
                out=ex[sl, :], in_=Sv, func=mybir.ActivationFunctionType.Exp,
                bias=nm[sl, :], scale=1.0,
            )
            ssum = smallp.tile([36, 1], f32, tag=f"ssum{gi}")
            nc.vector.tensor_reduce(
                out=ssum[sl, :], in_=ex[sl, :], axis=mybir.AxisListType.X,
                op=mybir.AluOpType.add,
            )
            rec = smallp.tile([36, 1], f32, tag=f"rec{gi}")
            nc.vector.reciprocal(out=rec[sl, :], in_=ssum[sl, :])
            wm = smallp.tile([36, T], f32, tag=f"wm{gi}")
            nc.vector.tensor_tensor(
                out=wm[sl, :], in0=ex[sl, :], in1=mask_sb[sl, :],
                op=mybir.AluOpType.mult,
            )
            # w rows land in wfp at partitions 0-3 (A) / 32-35 (B); oh_rows
            # places each row's one-hot at the matching partition
            nc.vector.tensor_scalar(
                out=wfp[sl, :], in0=wm[sl, :], scalar1=rec[sl, :],
                scalar2=None, op0=mybir.AluOpType.mult,
            )

            # v^T[e, b] = sum_t x^T[e, t] * w[b, t]: broadcast w row across
            # partitions via K=128 one-hot matmul, then mult+reduce on DVE
            for b in rows:
                wb = p_x.tile([128, T], f32, tag="x")  # reuse x psum slots
                nc.tensor.matmul(
                    out=wb[:],
                    lhsT=ohr_sb[:, b * 128:(b + 1) * 128],
                    rhs=wfp[:],
                    start=True,
                    stop=True,
                )
                wb16 = scrp.tile([128, T], f16, tag="wb16")
                nc.vector.tensor_copy(out=wb16[:], in_=wb[:])
                scr = scrp.tile([128, T], f16, tag="scr")
                nc.vector.tensor_tensor(
                    out=scr[:], in0=xt_tiles[b][:], in1=wb16[:],
                    op=mybir.AluOpType.mult,
                )
                nc.vector.tensor_reduce(
                    out=vt_sb[:, b:b + 1], in_=scr[:],
                    axis=mybir.AxisListType.X, op=mybir.AluOpType.add,
                )
        vt16 = smallp.tile([128, BC], f16, tag="vt16")
        nc.vector.tensor_copy(out=vt16[:], in_=vt_sb[:])

        # ---- out = v @ out_W.T + out_b ----  (one PSUM bank per matmul)
        o_sb = smallp.tile([BC, OUT], f32, tag="o")
        po_a = p_s.tile([BC, 512], f32, tag="poa")
        nc.tensor.matmul(
            out=po_a[:], lhsT=vt16[:], rhs=owt_sb[:, 0:512],
            start=True, stop=True,
        )
        nc.vector.tensor_tensor(
            out=o_sb[:, 0:512], in0=po_a[:], in1=ob_sb[:, 0:512],
            op=mybir.AluOpType.add,
        )
        po_b = p_s.tile([BC, OUT - 512], f32, tag="pob")
        nc.tensor.matmul(
            out=po_b[:], lhsT=vt16[:], rhs=owt_sb[:, 512:OUT],
            start=True, stop=True,
        )
        nc.vector.tensor_tensor(
            out=o_sb[:, 512:OUT], in0=po_b[:], in1=ob_sb[:, 512:OUT],
            op=mybir.AluOpType.add,
        )
        nc.sync.dma_start(out=d_out[:], in_=o_sb[:])

    nc.compile()
    return nc


def _get_built():
    global _BUILT
    if _BUILT is None:
        _BUILT = _build()
    return _BUILT


def _prep_shared(node_emb, path_emb, W, a, out_W, out_b):
    node_z = np.empty((NODES + 1, E), dtype=np.float16)
    node_z[:NODES] = np.asarray(node_emb, dtype=np.float32).astype(np.float16)
    node_z[PAD, :] = 0.0
    node_z[NODES] = 0.0
    path_z = np.empty((PATHS + 1, E), dtype=np.float16)
    path_z[:PATHS] = np.asarray(path_emb, dtype=np.float32).astype(np.float16)
    path_z[PATHS] = 0.0
    # wt[d, k, e] = W[e, 128k + d]
    wt = np.ascontiguousarray(
        np.asarray(W, dtype=np.float32).reshape(E, 3, E).transpose(2, 1, 0)
    ).astype(np.float16)
    a_oh = np.zeros((E, BC * BC), dtype=np.float16)
    for b in range(BC):
        a_oh[:, b * BC + b] = np.asarray(a, dtype=np.float32).astype(np.float16)
    # w row for batch-row b lives in wfp at partition b (rows 0-3) or
    # b+28 (rows 4-7, group B at base partition 32)
    oh_rows = np.zeros((128, BC * 128), dtype=np.float16)
    for b in range(BC):
        pb = b if b < 4 else b + 28
        oh_rows[pb, b * 128:(b + 1) * 128] = 1.0
    owt = np.ascontiguousarray(
        np.asarray(out_W, dtype=np.float32).T
    ).astype(np.float16)
    ob = np.ascontiguousarray(
        np.broadcast_to(np.asarray(out_b, dtype=np.float32), (BC, OUT))
    )
    return node_z, path_z, wt, a_oh, oh_rows, owt, ob


def _idx_tile(idx_rows):
    # [BC, T] -> [128, J] with tile[p, 4b+c] = idx_rows[b, 128c + p]
    return np.ascontiguousarray(
        np.asarray(idx_rows).reshape(BC, CHUNKS, 128).transpose(2, 0, 1)
        .reshape(128, J).astype(np.int32)
    )


def make_in_maps(starts, paths, ends, length, node_emb, path_emb, W, a, out_W, out_b):
    node_z, path_z, wt, a_oh, oh_rows, owt, ob = _prep_shared(
        node_emb, path_emb, W, a, out_W, out_b
    )
    length = np.asarray(length)
    in_maps = []
    # Per-row token order is free (softmax over full T and the weighted sum
    # are permutation-invariant once the mask is permuted along): sort each
    # row's tokens by starts-index so the node-table gathers hit ascending
    # HBM addresses (better row/bank locality on 1/3 of gather traffic).
    starts = np.asarray(starts)
    paths = np.asarray(paths)
    ends = np.asarray(ends)
    perm = np.argsort(starts, axis=1, kind="stable")
    rix = np.arange(B)[:, None]
    starts = starts[rix, perm]
    paths = paths[rix, perm]
    ends = ends[rix, perm]
    for k in range(NCORES):
        rows = slice(k * BC, (k + 1) * BC)
        mask8 = (
            perm[rows] < np.asarray(length[rows])[:, None]
        ).astype(np.float32)
        mask = np.zeros((36, T), dtype=np.float32)
        mask[0:4] = mask8[0:4]
        mask[32:36] = mask8[4:8]
        in_maps.append(dict(
            s_idx=_idx_tile(starts[rows]),
            p_idx=_idx_tile(paths[rows]),
            e_idx=_idx_tile(ends[rows]),
            node_emb=node_z,
            path_emb=path_z,
            wt=wt,
            a_oh=a_oh,
            oh_rows=oh_rows,
            mask=np.ascontiguousarray(mask),
            out_wt=owt,
            out_b=ob,
            ident=np.eye(128, dtype=np.float16),
        ))
    return in_maps


def kernel(starts, paths, ends, length, node_emb, path_emb, W, a, out_W, out_b):
    global LAST_RESULTS
    import os

    if not TRACE:
        # trace=True needs antenv.axon_hooks, absent on this image; make sure
        # an ambient BASS_TRACE can't route us into that path
        os.environ["BASS_NEVER_TRACE"] = "1"
    from concourse.bass_utils import run_bass_kernel_spmd

    nc = _get_built()
    in_maps = make_in_maps(
        starts, paths, ends, length, node_emb, path_emb, W, a, out_W, out_b
    )
    res = run_bass_kernel_spmd(
        nc, in_maps, core_ids=list(range(NCORES)), trace=TRACE
    )
    LAST_RESULTS = res
    return np.concatenate([r["out"] for r in res.results], axis=0)
